# revision 2
# baseline (speedup 1.0000x reference)
import os
import time
import numpy as np

# DCRNN (nn_DCRNNModel): N=1024 nodes, U=64, K=2, S=2 supports, Q=5,
# B=64, T=12, H=12, IN=OUT=1, L=2 layers.
# Strategy: data-parallel over batch (B=64 -> 8 per NeuronCore), supports
# and weights replicated. The whole recurrent model is jitted with XLA on
# the 8 axon-tunneled NeuronCores via pmap. The Chebyshev diffusion
# recurrence x2 = 2*A@(A@z) - z is restructured as a single stacked GEMM
# with D = [A0; 2*A0^2-I; A1; 2*A1^2-I] (exact same linear map), which
# turns each cell's message passing into one (4N, N) x (N, B*F) matmul.
N, U, K, S = 1024, 64, 2, 2
Q = S * K + 1
B, T, H = 64, 12, 12
IN, OUT, L = 1, 1, 2

LAST_EXEC_NS = 0

# Persist the neuron compile cache across harness invocations.
os.environ.setdefault("NEURON_CC_FLAGS", "--cache_dir=/var/tmp/neuron-compile-cache")


def _build_jax_model():
    import jax
    import jax.numpy as jnp

    def diffusion(z, D):
        # z: (b, N, F); D: (4N, N) stacked [A0, M0, A1, M1], M=2A^2-I
        b, n, f = z.shape
        y = jnp.einsum('pn,bnf->bpf', D, z)          # (b, 4N, F)
        y = y.reshape(b, 4, n, f)
        full = jnp.concatenate([z[:, None], y], axis=1)   # (b, 5, N, F)
        return full.transpose(0, 2, 1, 3).reshape(b, n, Q * f)

    def cell(x, h, D, gw, gb, cw, cb):
        ru = jax.nn.sigmoid(diffusion(jnp.concatenate([x, h], -1), D) @ gw + gb)
        r, u = ru[..., :U], ru[..., U:]
        c = jnp.tanh(diffusion(jnp.concatenate([x, r * h], -1), D) @ cw + cb)
        return u * h + (1.0 - u) * c

    def percore(inputs, supports, enc, dec, proj_w, proj_b):
        # inputs: (b, N*IN, T) local batch shard
        b = inputs.shape[0]
        A0, A1 = supports[0], supports[1]
        eye = jnp.eye(N, dtype=jnp.float32)
        D = jnp.concatenate(
            [A0, 2.0 * (A0 @ A0) - eye, A1, 2.0 * (A1 @ A1) - eye], axis=0)

        x_seq = inputs.reshape(b, N, IN, T).transpose(3, 0, 1, 2)
        h0 = jnp.zeros((L, b, N, U), jnp.float32)

        def enc_step(h, x):
            out, hs = x, []
            for l in range(L):
                hn = cell(out, h[l], D, *enc[l])
                hs.append(hn)
                out = hn
            return jnp.stack(hs), None

        hT, _ = jax.lax.scan(enc_step, h0, x_seq)

        def dec_step(carry, _):
            h, xin = carry
            out, hs = xin, []
            for l in range(L):
                hn = cell(out, h[l], D, *dec[l])
                hs.append(hn)
                out = hn
            proj = out @ proj_w + proj_b
            return (jnp.stack(hs), proj), proj.reshape(b, N * OUT)

        go = jnp.zeros((b, N, OUT), jnp.float32)
        _, outs = jax.lax.scan(dec_step, (hT, go), None, length=H)
        return outs  # (H, b, N*OUT)

    return percore


def _run_jax_axon(inputs, supports, enc, dec, proj_w, proj_b):
    global LAST_EXEC_NS
    import jax

    devs = [d for d in jax.devices() if d.platform != "cpu"][:8]
    if len(devs) < 8:
        raise RuntimeError("need 8 accelerator devices")

    percore = _build_jax_model()
    pm = jax.pmap(
        percore,
        in_axes=(0, None, None, None, None, None),
        devices=devs,
    )
    xin = inputs.reshape(8, B // 8, N * IN, T)
    out = pm(xin, supports, enc, dec, proj_w, proj_b)
    out.block_until_ready()
    t0 = time.time()
    out = pm(xin, supports, enc, dec, proj_w, proj_b)
    out.block_until_ready()
    LAST_EXEC_NS = int((time.time() - t0) * 1e9)
    out = np.asarray(out)  # (8, H, B/8, N*OUT)
    return out.transpose(1, 0, 2, 3).reshape(H, B, N * OUT).astype(np.float32)


def _sigmoid(x):
    return 1.0 / (1.0 + np.exp(-x))


def _run_numpy(inputs, supports, enc, dec, proj_w, proj_b):
    # BLAS-restructured fallback: one (4N,N)x(N,B*F) GEMM per diffusion.
    A0, A1 = supports[0], supports[1]
    eye = np.eye(N, dtype=np.float32)
    D = np.concatenate(
        [A0, 2.0 * (A0 @ A0) - eye, A1, 2.0 * (A1 @ A1) - eye], axis=0)

    def diffusion(z):
        b, n, f = z.shape
        z2 = z.transpose(1, 0, 2).reshape(n, b * f)
        y = D @ z2                                   # (4N, b*f)
        y = y.reshape(4, n, b, f).transpose(2, 1, 0, 3)   # (b, N, 4, F)
        full = np.concatenate([z[:, :, None, :], y], axis=2)
        return full.reshape(b, n, Q * f)

    def cell(x, h, gw, gb, cw, cb):
        ru = _sigmoid(diffusion(np.concatenate([x, h], -1)) @ gw + gb)
        r, u = ru[..., :U], ru[..., U:]
        c = np.tanh(diffusion(np.concatenate([x, r * h], -1)) @ cw + cb)
        return u * h + (1.0 - u) * c

    x_seq = inputs.reshape(B, N, IN, T).transpose(3, 0, 1, 2)
    h = [np.zeros((B, N, U), np.float32) for _ in range(L)]
    for t in range(T):
        out = x_seq[t]
        for l in range(L):
            h[l] = cell(out, h[l], *enc[l])
            out = h[l]
    outs = []
    xin = np.zeros((B, N, OUT), np.float32)
    for _ in range(H):
        out = xin
        for l in range(L):
            h[l] = cell(out, h[l], *dec[l])
            out = h[l]
        proj = out @ proj_w + proj_b
        outs.append(proj.reshape(B, N * OUT))
        xin = proj
    return np.stack(outs).astype(np.float32)


def kernel(inputs, supports,
           enc_gw0, enc_gb0, enc_cw0, enc_cb0,
           enc_gw1, enc_gb1, enc_cw1, enc_cb1,
           dec_gw0, dec_gb0, dec_cw0, dec_cb0,
           dec_gw1, dec_gb1, dec_cw1, dec_cb1,
           proj_w, proj_b):
    enc = ((np.asarray(enc_gw0, np.float32), np.asarray(enc_gb0, np.float32),
            np.asarray(enc_cw0, np.float32), np.asarray(enc_cb0, np.float32)),
           (np.asarray(enc_gw1, np.float32), np.asarray(enc_gb1, np.float32),
            np.asarray(enc_cw1, np.float32), np.asarray(enc_cb1, np.float32)))
    dec = ((np.asarray(dec_gw0, np.float32), np.asarray(dec_gb0, np.float32),
            np.asarray(dec_cw0, np.float32), np.asarray(dec_cb0, np.float32)),
           (np.asarray(dec_gw1, np.float32), np.asarray(dec_gb1, np.float32),
            np.asarray(dec_cw1, np.float32), np.asarray(dec_cb1, np.float32)))
    inputs = np.ascontiguousarray(np.asarray(inputs, np.float32))
    supports = np.ascontiguousarray(np.asarray(supports, np.float32))
    proj_w = np.asarray(proj_w, np.float32)
    proj_b = np.asarray(proj_b, np.float32)
    try:
        return _run_jax_axon(inputs, supports, enc, dec, proj_w, proj_b)
    except Exception:
        return _run_numpy(inputs, supports, enc, dec, proj_w, proj_b)
